# revision 1
# baseline (speedup 1.0000x reference)
"""Trainium2 Bass kernel for multi-head attention (B=4, C=256, N=4096, H=4).

Sharding: 16 (batch, head) pairs across 8 cores -> each core computes 2 heads
of one batch entirely locally (no collectives). The final projection is
column-separable over heads, so each core emits a partial [C, N] output and
the host sums the two partials per batch plus bias/residual terms.

Per-core pipeline (all matmuls bf16, accumulation f32 in PSUM):
  q2/k2 = W[2 heads] @ x          [128, N]  (q gets +bq and *1/sqrt(dk) folded)
  vT    = x^T @ WvT               [N, 128] tiles, with ones columns appended
  S^T   = k2^T q2 per (128 key x 512 query) tile, 2 heads row-packed in PE
  P     = exp(S^T) on ScalarE (no max subtraction; scores ~ N(0,1))
  pv    = [v | 1]^T P  -> attention numerator + denominator row via ones col
  at    = pv[:64] * (1/pv[64])    broadcast via DMA
  out   = WpT @ at  (partial final projection)

Bias folds: bk cancels exactly in softmax (constant along key axis);
bv folds into a host-side constant (attention rows sum to 1); bq on device.
"""

import sys

import numpy as np
import ml_dtypes

if "/opt/trn_rl_repo" not in sys.path:
    sys.path.insert(0, "/opt/trn_rl_repo")

B, C, N, H = 4, 256, 4096, 4
DK = 64
DD = 128          # 2 heads * DK
NB = 512          # query block
NBLK = N // NB    # 8
MT = 128          # key tile
MTILES = N // MT  # 32
MG = 1            # key tiles per PSUM group
NGRP = MTILES // MG
SCALE = 1.0 / np.sqrt(DK)

_NC_CACHE = {}


def build_nc():
    import concourse.bass as bass
    import concourse.mybir as mybir
    import concourse.tile as tile

    bf16 = mybir.dt.bfloat16
    f32 = mybir.dt.float32
    Exp = mybir.ActivationFunctionType.Exp
    Alu = mybir.AluOpType

    nc = bass.Bass(target_bir_lowering=False)

    x_d = nc.declare_dram_parameter("x", [C, N], bf16, isOutput=False)
    wqT_d = nc.declare_dram_parameter("wqT", [C, DD], bf16, isOutput=False)
    wkT_d = nc.declare_dram_parameter("wkT", [C, DD], bf16, isOutput=False)
    wvT_d = nc.declare_dram_parameter("wvT", [C, DD], bf16, isOutput=False)
    wpT_d = nc.declare_dram_parameter("wpT", [2, DD, DD], bf16, isOutput=False)
    bq_d = nc.declare_dram_parameter("bq2", [DD, 1], f32, isOutput=False)
    out_d = nc.declare_dram_parameter("out", [C, N], f32, isOutput=True)

    with tile.TileContext(nc) as tc:
        with (
            tc.tile_pool(name="singles", bufs=1) as singles,
            tc.tile_pool(name="ppool", bufs=3) as ppool,
            tc.tile_pool(name="apool", bufs=3) as apool,
            tc.tile_pool(name="dpool", bufs=2, space="DRAM") as dpool,
            tc.tile_pool(name="psA", bufs=1, space="PSUM") as psA,
            tc.tile_pool(name="psS", bufs=3, space="PSUM") as psS,
            tc.tile_pool(name="psPV", bufs=4, space="PSUM") as psPV,
        ):
            # ---- load inputs ----
            x_sb = singles.tile([128, 2, N], bf16)
            nc.sync.dma_start(
                out=x_sb, in_=x_d[:, :].rearrange("(ko ki) n -> ki ko n", ki=128)
            )
            wq_sb = singles.tile([128, 2, DD], bf16)
            nc.sync.dma_start(
                out=wq_sb, in_=wqT_d[:, :].rearrange("(ko ki) m -> ki ko m", ki=128)
            )
            wk_sb = singles.tile([128, 2, DD], bf16)
            nc.sync.dma_start(
                out=wk_sb, in_=wkT_d[:, :].rearrange("(ko ki) m -> ki ko m", ki=128)
            )
            wv_sb = singles.tile([128, 2, DD], bf16)
            nc.sync.dma_start(
                out=wv_sb, in_=wvT_d[:, :].rearrange("(ko ki) m -> ki ko m", ki=128)
            )
            wp_sb = singles.tile([128, 2, DD], bf16)
            nc.sync.dma_start(
                out=wp_sb, in_=wpT_d[:, :, :].rearrange("j d m -> d j m")
            )
            bq_sb = singles.tile([128, 1], f32)
            nc.sync.dma_start(out=bq_sb, in_=bq_d[:, :])

            # Warm engine vector-clocks on one-time input DMAs so steady-state
            # instructions carry at most one semaphore wait (walrus limit).
            scr = singles.tile([128, 1], f32)
            nc.vector.tensor_copy(out=scr, in_=bq_sb)
            zero_sb = singles.tile([128, 1], f32)
            nc.vector.memset(zero_sb, 0.0)
            scr_out = singles.tile([128, 1], f32)
            nc.scalar.activation(out=scr_out, in_=scr, func=Exp, bias=zero_sb)
            # PE: observe each input-DMA queue once (dummy weight loads)
            nc.tensor.ldweights(x_sb[:, 0, 0:128])
            nc.tensor.ldweights(wq_sb[:, 0, :])
            nc.tensor.ldweights(wk_sb[:, 0, :])
            nc.tensor.ldweights(wv_sb[:, 0, :])
            nc.tensor.ldweights(wp_sb[:, 0, :])

            # ---- projections ----
            # Order: k2 fully, then vte, then q2 per block - attention block
            # nb only needs full k2/vte plus q2[:, nb], so q2 projections of
            # later blocks overlap the first attention blocks.
            q2 = singles.tile([128, N], bf16)
            k2 = singles.tile([128, N], bf16)
            for nb in range(NBLK):
                nsl = slice(nb * NB, (nb + 1) * NB)
                psk = psA.tile([128, NB], f32, tag="psA")
                for ko in range(2):
                    nc.tensor.matmul(
                        psk, wk_sb[:, ko, :], x_sb[:, ko, nsl],
                        start=(ko == 0), stop=(ko == 1), skip_group_check=True,
                    )
                nc.vector.tensor_copy(out=k2[:, nsl], in_=psk)

            # vT with ones columns: [64 v_h0 | 1 | 64 v_h1 | 1]
            vte = singles.tile([128, MTILES, 130], bf16)
            nc.vector.memset(vte[:, :, 64:65], 1.0)
            nc.vector.memset(vte[:, :, 129:130], 1.0)
            for g4 in range(MTILES // 4):
                psv = psA.tile([128, NB], f32, tag="psA")
                for dj in range(4):
                    j = g4 * 4 + dj
                    msl = slice(j * MT, (j + 1) * MT)
                    for ko in range(2):
                        nc.tensor.matmul(
                            psv[:, dj * 128:(dj + 1) * 128],
                            x_sb[:, ko, msl], wv_sb[:, ko, :],
                            start=(ko == 0), stop=(ko == 1), skip_group_check=True,
                        )
                psv3 = psv.rearrange("p (j c) -> p j c", c=128)
                nc.vector.tensor_copy(
                    out=vte[:, g4 * 4:(g4 + 1) * 4, 0:64], in_=psv3[:, :, 0:64]
                )
                nc.vector.tensor_copy(
                    out=vte[:, g4 * 4:(g4 + 1) * 4, 65:129], in_=psv3[:, :, 64:128]
                )

            for nb in range(NBLK):
                nsl = slice(nb * NB, (nb + 1) * NB)
                psq = psA.tile([128, NB], f32, tag="psA")
                for ko in range(2):
                    nc.tensor.matmul(
                        psq, wq_sb[:, ko, :], x_sb[:, ko, nsl],
                        start=(ko == 0), stop=(ko == 1), skip_group_check=True,
                    )
                # q2 = (Wq x + bq) * scale
                nc.vector.tensor_scalar(
                    q2[:, nsl], psq, bq_sb, float(SCALE), Alu.add, Alu.mult
                )

            # PE: observe the DVE tick that finished vte
            nc.tensor.ldweights(vte[:, MTILES - 1, 0:65])

            # ---- attention ----
            import os as _os
            nblk_run = int(_os.environ.get("KERNEL_NBLK", NBLK))
            at2_prev = None
            for nb in range(nblk_run):
                nsl = slice(nb * NB, (nb + 1) * NB)
                if at2_prev is not None:
                    # PE: observe DVE's at2 tick of the previous block so the
                    # first PV matmul below carries only the ACT wait
                    nc.tensor.ldweights(at2_prev[:, 0:128])
                pv0 = psPV.tile([128, NB], f32, tag="pv")
                pv1 = psPV.tile([128, NB], f32, tag="pv")
                for jg in range(NGRP):
                    ps0 = psS.tile([128, MG * NB], f32, tag="s")
                    ps1 = psS.tile([128, MG * NB], f32, tag="s")
                    for dj in range(MG):
                        j = jg * MG + dj
                        msl = slice(j * MT, (j + 1) * MT)
                        dsl = slice(dj * NB, (dj + 1) * NB)
                        nc.tensor.matmul(
                            ps0[:, dsl], k2[0:64, msl], q2[0:64, nsl],
                            start=True, stop=True, tile_position=(0, 0),
                            skip_group_check=True,
                        )
                        nc.tensor.matmul(
                            ps1[:, dsl], k2[64:128, msl], q2[64:128, nsl],
                            start=True, stop=True, tile_position=(64, 0),
                            skip_group_check=True,
                        )
                    pt0 = ppool.tile([128, MG * NB], bf16, tag="pt")
                    nc.scalar.activation(pt0, ps0, Exp, bias=zero_sb)
                    pt1 = ppool.tile([128, MG * NB], bf16, tag="pt")
                    if jg % 4 == 3:
                        # exp via Schraudolph bitcast: bf16 bits =
                        # round(s*128*log2e + (127<<7) - c); offloads ACT
                        nc.vector.tensor_scalar(
                            pt1.bitcast(mybir.dt.int16), ps1,
                            184.6650085511249, 16250.4, Alu.mult, Alu.add
                        )
                    else:
                        nc.scalar.activation(pt1, ps1, Exp, bias=zero_sb)
                    for dj in range(MG):
                        j = jg * MG + dj
                        dsl = slice(dj * NB, (dj + 1) * NB)
                        first = jg == 0 and dj == 0
                        last = jg == NGRP - 1 and dj == MG - 1
                        nc.tensor.matmul(
                            pv0[0:65, :], vte[:, j, 0:65], pt0[:, dsl],
                            start=first, stop=last, skip_group_check=True,
                        )
                        nc.tensor.matmul(
                            pv1[0:65, :], vte[:, j, 65:130], pt1[:, dsl],
                            start=first, stop=last, skip_group_check=True,
                        )

                # softmax denominators -> divide
                den = apool.tile([128, 2 * NB], f32, tag="den")
                nc.vector.reciprocal(out=den[64:65, 0:NB], in_=pv0[64:65, :])
                nc.vector.reciprocal(out=den[64:65, NB:2 * NB], in_=pv1[64:65, :])
                dd = dpool.tile([2, NB], f32, tag="dd")
                nc.gpsimd.dma_start(out=dd[0:1, :], in_=den[64:65, 0:NB])
                nc.gpsimd.dma_start(out=dd[1:2, :], in_=den[64:65, NB:2 * NB])
                bc = apool.tile([128, NB], f32, tag="bc")
                nc.gpsimd.dma_start(
                    out=bc[0:64, :], in_=dd[0:1, :].to_broadcast((64, NB))
                )
                nc.gpsimd.dma_start(
                    out=bc[64:128, :], in_=dd[1:2, :].to_broadcast((64, NB))
                )
                at2 = apool.tile([128, NB], bf16, tag="at")
                nc.vector.tensor_tensor(at2[0:64, :], pv0[0:64, :], bc[0:64, :], Alu.mult)
                nc.vector.tensor_tensor(at2[64:128, :], pv1[0:64, :], bc[64:128, :], Alu.mult)

                # partial final projection
                for oh in range(2):
                    pso = psA.tile([128, NB], f32, tag="psA")
                    nc.tensor.matmul(
                        pso, wp_sb[:, oh, :], at2,
                        start=True, stop=True, skip_group_check=True,
                    )
                    osb = apool.tile([128, NB], f32, tag="osb")
                    nc.scalar.activation(
                        out=osb, in_=pso, func=mybir.ActivationFunctionType.Copy
                    )
                    nc.gpsimd.dma_start(
                        out=out_d[:, :][oh * 128:(oh + 1) * 128, nsl], in_=osb
                    )
                at2_prev = at2
    return nc


def split_multiwaits(nc):
    """The staged walrus accepts at most one sync-wait per instruction; Tile
    emits several. Hoist all but one wait onto same-engine NOPs placed just
    before the instruction (engine program order makes this equivalent)."""
    import concourse.mybir as mybir

    n = 0
    for fn in nc.m.functions:
        for blk in fn.blocks:
            new = []
            for inst in blk.instructions:
                si = getattr(inst, "sync_info", None)
                waits = list(si.on_wait) if si is not None and si.on_wait else []
                if len(waits) > 1:
                    for k, w in enumerate(waits[:-1]):
                        new.append(mybir.InstNoOp(
                            name=f"{inst.name}-w{k}",
                            engine=inst.engine,
                            ins=[], outs=[],
                            sync_info=mybir.SyncInfo(on_wait=[w], on_update=[]),
                        ))
                        n += 1
                    inst.sync_info = mybir.SyncInfo(
                        on_wait=[waits[-1]], on_update=list(si.on_update)
                    )
                new.append(inst)
            blk.instructions = new
    return n


def _get_nc():
    if "nc" not in _NC_CACHE:
        nc = build_nc()
        split_multiwaits(nc)
        _NC_CACHE["nc"] = nc
    return _NC_CACHE["nc"]


def _make_in_maps(x, wq, bq, wk, wv, wp):
    bf = ml_dtypes.bfloat16
    in_maps = []
    for core in range(8):
        b = core // 2
        hp = core % 2
        rs = slice(hp * DD, (hp + 1) * DD)
        in_maps.append({
            "x": np.ascontiguousarray(x[b]).astype(bf),
            "wqT": np.ascontiguousarray(wq[rs, :].T).astype(bf),
            "wkT": np.ascontiguousarray(wk[rs, :].T).astype(bf),
            "wvT": np.ascontiguousarray(wv[rs, :].T).astype(bf),
            "wpT": np.stack(
                [np.ascontiguousarray(wp[j * DD:(j + 1) * DD, rs].T) for j in range(2)]
            ).astype(bf),
            "bq2": np.ascontiguousarray(bq[rs]).reshape(DD, 1).astype(np.float32),
        })
    return in_maps


def run(x, wq, bq, wk, bk, wv, bv, wp, bp, trace=False):
    from concourse.bass_utils import run_bass_kernel_spmd

    x = np.asarray(x, dtype=np.float32)
    wq = np.asarray(wq, dtype=np.float32)
    bq = np.asarray(bq, dtype=np.float32)
    wk = np.asarray(wk, dtype=np.float32)
    wv = np.asarray(wv, dtype=np.float32)
    bv = np.asarray(bv, dtype=np.float32)
    wp = np.asarray(wp, dtype=np.float32)
    bp = np.asarray(bp, dtype=np.float32)

    nc = _get_nc()
    in_maps = _make_in_maps(x, wq, bq, wk, wv, wp)
    res = run_bass_kernel_spmd(nc, in_maps, core_ids=list(range(8)), trace=trace)
    parts = [r["out"].astype(np.float32) for r in res.results]

    const = (bp + wp @ bv).astype(np.float32)[:, None]  # [C, 1]
    out = np.empty((B, C, N), dtype=np.float32)
    for b in range(B):
        out[b] = parts[2 * b] + parts[2 * b + 1] + x[b] + const
    return out, res


def kernel(**inputs):
    out, _ = run(**inputs)
    return out



# revision 4
# speedup vs baseline: 232.8689x; 232.8689x over previous
"""Trainium2 Bass kernel v2 for multi-head attention (B=4, C=256, N=4096, H=4).

Sharding: 16 (batch, head) pairs across 8 cores -> each core computes 2 heads
of one batch locally (no collectives). Host sums two partial [C,N] outputs
per batch plus bias/residual terms.

v2 changes vs baseline (sim: ACT 80% busy was the bottleneck, DVE 23% idle):
  - exp split ~56/44 between ScalarE (true exp) and VectorE (Schraudolph via
    uint8 bitcast), one FD=1024 instruction per 128-key tile covering both
    heads (amortizes per-instr overhead).
  - P stored in fp8e4 with a global exponent shift (P = exp(s-2), cancels in
    softmax); PV matmul uses fp8 DoubleRow (virtual K=256) halving PV time.
  - software pipelining: PV matmuls lag scores by 2 chunks; block nb's output
    projection runs inside block nb+1's score loop (PE never waits on exp or
    the softmax-divide tail).
  - single PSUM tag ring [128,2,512]x3 shared by scores/prologue/projection
    (6 banks) + 2 PV accumulator banks = 8.
  - bf16 tails (pv drain, reciprocal, 1/den broadcast, divide, output).
"""

import sys

import numpy as np
import ml_dtypes

if "/opt/trn_rl_repo" not in sys.path:
    sys.path.insert(0, "/opt/trn_rl_repo")

B, C, N, H = 4, 256, 4096, 4
DK = 64
DD = 128          # 2 heads * DK
NB = 512          # query block
NBLK = N // NB    # 8
MT = 128          # key tile
MTILES = N // MT  # 32
NCHUNK = MTILES // 2  # 16 x 256-key chunks for DoubleRow PV
SCALE = 1.0 / np.sqrt(DK)
SHIFT = 2.0       # P = exp(s - SHIFT); cancels in softmax
LOG2E = 1.4426950408889634
# DVE Schraudolph to fp8e4 bits: bits = s*8*log2e + (56 - 8*SHIFT*log2e - c)
SCH_MUL = 8.0 * LOG2E
SCH_ADD = 56.0 - 8.0 * SHIFT * LOG2E - 0.5

_NC_CACHE = {}


def _exp_on_act(j):
    """Engine split per 2-tile chunk (both subs on one engine so the PV
    matmul carries a single cross-engine wait): 18 ACT / 14 DVE per 32."""
    ch = j // 2
    return (ch % 2 == 0) or (ch % 16 == 15)


def build_nc(reps=1, debug=False):
    import concourse.bass as bass
    import concourse.mybir as mybir
    import concourse.tile as tile

    bf16 = mybir.dt.bfloat16
    f32 = mybir.dt.float32
    fp8 = mybir.dt.float8e4
    u8 = mybir.dt.uint8
    Exp = mybir.ActivationFunctionType.Exp
    Copy = mybir.ActivationFunctionType.Copy
    Alu = mybir.AluOpType
    DR = mybir.MatmulPerfMode.DoubleRow

    nc = bass.Bass(target_bir_lowering=False)

    x_d = nc.declare_dram_parameter("x", [C, N], bf16, isOutput=False)
    wqT_d = nc.declare_dram_parameter("wqT", [C, DD], bf16, isOutput=False)
    wkT_d = nc.declare_dram_parameter("wkT", [C, DD], bf16, isOutput=False)
    wvT_d = nc.declare_dram_parameter("wvT", [C, DD], bf16, isOutput=False)
    wpT_d = nc.declare_dram_parameter("wpT", [2, 2, 64, DD], bf16, isOutput=False)
    bq_d = nc.declare_dram_parameter("bq2", [DD, 1], f32, isOutput=False)
    out_d = nc.declare_dram_parameter("out", [C, N], bf16, isOutput=True)
    if debug:
        dq2_d = nc.declare_dram_parameter("dq2", [128, N], bf16, isOutput=True)
        dk2_d = nc.declare_dram_parameter("dk2", [128, N], bf16, isOutput=True)
        dvt_d = nc.declare_dram_parameter("dvt", [128, NCHUNK * 2 * 2 * 80],
                                          mybir.dt.uint8, isOutput=True)
        dp8_d = nc.declare_dram_parameter("dp8", [128, 2 * 2 * NB],
                                          mybir.dt.uint8, isOutput=True)
        dpv_d = nc.declare_dram_parameter("dpv", [128, 2 * NB], bf16,
                                          isOutput=True)
        drs_d = nc.declare_dram_parameter("drs", [1, 2 * NB], bf16, isOutput=True)
        dbc_d = nc.declare_dram_parameter("dbc", [64, 2 * NB], bf16, isOutput=True)
        dat_d = nc.declare_dram_parameter("dat", [64, 2 * NB], bf16, isOutput=True)

    with tile.TileContext(nc) as tc:
        with (
            tc.tile_pool(name="singles", bufs=1) as singles,
            tc.tile_pool(name="ppool", bufs=4) as ppool,
            tc.tile_pool(name="apool", bufs=3) as apool,
            tc.tile_pool(name="dpool", bufs=2, space="DRAM") as dpool,
            tc.tile_pool(name="psS", bufs=3, space="PSUM") as psS,
            tc.tile_pool(name="psPV", bufs=2, space="PSUM") as psPV,
        ):
            # ---- load inputs ----
            x_sb = singles.tile([128, 2, N], bf16)
            for xc in range(8):
                xsl = slice(xc * (N // 8), (xc + 1) * (N // 8))
                nc.sync.dma_start(
                    out=x_sb[:, :, xsl],
                    in_=x_d[:, xsl].rearrange("(ko ki) n -> ki ko n", ki=128),
                )
            wq_sb = singles.tile([128, 2, DD], bf16)
            nc.sync.dma_start(
                out=wq_sb, in_=wqT_d[:, :].rearrange("(ko ki) m -> ki ko m", ki=128)
            )
            wk_sb = singles.tile([128, 2, DD], bf16)
            nc.sync.dma_start(
                out=wk_sb, in_=wkT_d[:, :].rearrange("(ko ki) m -> ki ko m", ki=128)
            )
            wv_sb = singles.tile([128, 2, DD], bf16)
            nc.sync.dma_start(
                out=wv_sb, in_=wvT_d[:, :].rearrange("(ko ki) m -> ki ko m", ki=128)
            )
            wp_sb = singles.tile([64, 2, 2, DD], bf16)
            nc.sync.dma_start(
                out=wp_sb, in_=wpT_d[:, :, :, :].rearrange("hp j d m -> d hp j m")
            )
            bq_sb = singles.tile([128, 1], f32)
            nc.sync.dma_start(out=bq_sb, in_=bq_d[:, :])

            # Warm engine vector-clocks on one-time input DMAs so steady-state
            # instructions carry at most one semaphore wait (walrus limit).
            scr = singles.tile([128, 1], f32)
            nc.vector.tensor_copy(out=scr, in_=bq_sb)
            nbias_sb = singles.tile([128, 1], f32)
            nc.vector.memset(nbias_sb, -float(SHIFT))
            scr_out = singles.tile([128, 1], f32)
            nc.scalar.activation(out=scr_out, in_=scr, func=Exp, bias=nbias_sb)
            nc.tensor.ldweights(x_sb[:, 0, 0:128])
            nc.tensor.ldweights(wq_sb[:, 0, :])
            nc.tensor.ldweights(wk_sb[:, 0, :])
            nc.tensor.ldweights(wv_sb[:, 0, :])
            nc.tensor.ldweights(wp_sb[:, 0, 0, :])

            import contextlib as _ctxlib
            with (tc.For_i(0, reps, 1) if reps > 1 else _ctxlib.nullcontext()):
              if True:
                # ---- projections (prologue), all PSUM from the psS ring ----
                q2 = singles.tile([128, N], bf16)
                k2 = singles.tile([128, N], bf16)
                for np2 in range(NBLK // 2):   # 2 query blocks per psk tile
                    psk = psS.tile([128, 2, NB], f32, tag="s")
                    for t in range(2):
                        nsl = slice((np2 * 2 + t) * NB, (np2 * 2 + t + 1) * NB)
                        for ko in range(2):
                            nc.tensor.matmul(
                                psk[:, t, :], wk_sb[:, ko, :], x_sb[:, ko, nsl],
                                start=(ko == 0), stop=(ko == 1),
                                skip_group_check=True,
                            )
                    nc.scalar.activation(
                        out=k2[:, np2 * 2 * NB:(np2 + 1) * 2 * NB], in_=psk,
                        func=Copy,
                    )

                # v in fp8, transposed: vte8[key_part, chunk, head, sub, col]
                # col 0:64 = v, col 64 = ones (-> softmax denominator row)
                vte8 = singles.tile([128, NCHUNK, 2, 2, 80], fp8)
                nc.vector.memset(vte8[:, :, :, :, 64:65], 1.0)
                for g8 in range(MTILES // 8):   # 8 key tiles per psv tile
                    psv = psS.tile([128, 2, NB], f32, tag="s")
                    for dj in range(8):
                        j = g8 * 8 + dj
                        msl = slice(j * MT, (j + 1) * MT)
                        t, o = dj // 4, dj % 4
                        for ko in range(2):
                            nc.tensor.matmul(
                                psv[:, t, o * 128:(o + 1) * 128],
                                x_sb[:, ko, msl], wv_sb[:, ko, :],
                                start=(ko == 0), stop=(ko == 1),
                                skip_group_check=True,
                            )
                    # bank t holds tiles 8g+4t..8g+4t+3 = chunks 4g+2t..4g+2t+1
                    psv5 = psv.rearrange("p t (ch s c) -> p t ch s c",
                                         ch=2, s=2)
                    for h in range(2):
                        nc.vector.tensor_copy(
                            out=vte8[:, g8 * 4:(g8 + 1) * 4, h, :, 0:64],
                            in_=psv5[:, :, :, :, h * 64:(h + 1) * 64],
                        )

                for np2 in range(NBLK // 2):
                    psq = psS.tile([128, 2, NB], f32, tag="s")
                    for t in range(2):
                        nsl = slice((np2 * 2 + t) * NB, (np2 * 2 + t + 1) * NB)
                        for ko in range(2):
                            nc.tensor.matmul(
                                psq[:, t, :], wq_sb[:, ko, :], x_sb[:, ko, nsl],
                                start=(ko == 0), stop=(ko == 1),
                                skip_group_check=True,
                            )
                    # q2 = (Wq x + bq) * scale
                    nc.vector.tensor_scalar(
                        q2[:, np2 * 2 * NB:(np2 + 1) * 2 * NB], psq, bq_sb,
                        float(SCALE), Alu.add, Alu.mult
                    )

                # PE: observe the DVE tick that finished vte8
                nc.tensor.ldweights(vte8[:, NCHUNK - 1, 1, 0, 0:65])

                if debug:
                    nc.sync.dma_start(out=dq2_d[:, :], in_=q2)
                    nc.sync.dma_start(out=dk2_d[:, :], in_=k2)
                    nc.sync.dma_start(
                        out=dvt_d[:, :],
                        in_=vte8.bitcast(mybir.dt.uint8).rearrange(
                            "p a b c d -> p (a b c d)"),
                    )

                # ---- attention ----
                import os as _os
                nblk_run = int(_os.environ.get("KERNEL_NBLK", NBLK))
                PV_LAG = 2

                pend = {}         # block -> (pv0, pv1, {chunk: p8})
                proj_pend = None  # (at2, nb)

                def emit_scores(nb, ch):
                    nsl = slice(nb * NB, (nb + 1) * NB)
                    p8 = ppool.tile([128, 2, 2, NB], fp8, tag="p8")
                    for s2 in range(2):
                        j = ch * 2 + s2
                        msl = slice(j * MT, (j + 1) * MT)
                        ps = psS.tile([128, 2, NB], f32, tag="s")
                        nc.tensor.matmul(
                            ps[:, 0, :], k2[0:64, msl], q2[0:64, nsl],
                            start=True, stop=True, tile_position=(0, 0),
                            skip_group_check=True,
                        )
                        nc.tensor.matmul(
                            ps[:, 1, :], k2[64:128, msl], q2[64:128, nsl],
                            start=True, stop=True, tile_position=(64, 0),
                            skip_group_check=True,
                        )
                        if _exp_on_act(j):
                            nc.scalar.activation(
                                out=p8[:, :, s2, :], in_=ps, func=Exp,
                                bias=nbias_sb,
                            )
                        else:
                            nc.vector.tensor_scalar(
                                p8[:, :, s2, :].bitcast(u8), ps,
                                float(SCH_MUL), float(SCH_ADD),
                                Alu.mult, Alu.add,
                            )
                    if debug and nb == 0 and ch == 0:
                        nc.sync.dma_start(
                            out=dp8_d[:, :],
                            in_=p8.bitcast(u8).rearrange("p a b c -> p (a b c)"),
                        )
                    return p8

                def emit_pv(nb, ch, p8):
                    pv0, pv1 = pend[nb][0], pend[nb][1]
                    first = ch == 0
                    last = ch == NCHUNK - 1
                    nc.tensor.matmul(
                        pv0[0:65, :], vte8[:, ch, 0, :, 0:65], p8[:, 0, :, :],
                        start=first, stop=last, perf_mode=DR,
                        skip_group_check=True,
                    )
                    nc.tensor.matmul(
                        pv1[0:65, :], vte8[:, ch, 1, :, 0:65], p8[:, 1, :, :],
                        start=first, stop=last, perf_mode=DR,
                        skip_group_check=True,
                    )

                def emit_tail(nb):
                    # pv -> sbuf (frees PV banks), reciprocal of den row, then
                    # broadcast 1/den across partitions via SBUF->SBUF DMA
                    # (no PE involvement; latency hidden under next block).
                    pv0, pv1 = pend[nb][0], pend[nb][1]
                    pvsb = apool.tile([128, 2, NB], bf16, tag="pvsb")
                    nc.any.tensor_copy(out=pvsb[0:65, 0, :], in_=pv0[0:65, :])
                    nc.any.tensor_copy(out=pvsb[0:65, 1, :], in_=pv1[0:65, :])
                    rsb = apool.tile([1, 2, NB], bf16, tag="rsb")
                    with nc.allow_low_precision(
                        reason="softmax den reciprocal in bf16; validated "
                        "end-to-end rel err 1.2e-3 vs 2e-2 gate"
                    ):
                        nc.vector.reciprocal(out=rsb, in_=pvsb[64:65, :, :])
                    dd = dpool.tile([2, NB], bf16, tag="dd")
                    nc.sync.dma_start(out=dd[0:1, :], in_=rsb[0:1, 0, :])
                    nc.sync.dma_start(out=dd[1:2, :], in_=rsb[0:1, 1, :])
                    bc = apool.tile([64, 2, NB], bf16, tag="bc")
                    nc.sync.dma_start(
                        out=bc[:, 0, :], in_=dd[0:1, :].to_broadcast((64, NB))
                    )
                    nc.sync.dma_start(
                        out=bc[:, 1, :], in_=dd[1:2, :].to_broadcast((64, NB))
                    )
                    if debug and nb == 0:
                        nc.sync.dma_start(
                            out=drs_d[:, :],
                            in_=rsb.rearrange("p a b -> p (a b)"),
                        )
                        nc.sync.dma_start(
                            out=dbc_d[:, :],
                            in_=bc.rearrange("p a b -> p (a b)"),
                        )
                    return pvsb, bc

                def emit_at2(nb, pvsb, bc):
                    at2 = apool.tile([64, 2, NB], bf16, tag="at")
                    nc.vector.tensor_tensor(
                        at2, pvsb[0:64, :, :], bc, Alu.mult
                    )
                    if debug and nb == 0:
                        nc.sync.dma_start(
                            out=dpv_d[:, :],
                            in_=pvsb.rearrange("p a b -> p (a b)"),
                        )
                        nc.sync.dma_start(
                            out=dat_d[:, :],
                            in_=at2.rearrange("p a b -> p (a b)"),
                        )
                    return at2

                def emit_proj(at2, nb):
                    nsl = slice(nb * NB, (nb + 1) * NB)
                    pso = psS.tile([128, 2, NB], f32, tag="s")
                    for oh in range(2):
                        for hp in range(2):
                            nc.tensor.matmul(
                                pso[:, oh, :], wp_sb[:, hp, oh, :], at2[:, hp, :],
                                start=(hp == 0), stop=(hp == 1),
                                skip_group_check=True,
                            )
                    osb = apool.tile([128, 2, NB], bf16, tag="osb")
                    nc.any.tensor_copy(out=osb, in_=pso)
                    nc.sync.dma_start(
                        out=out_d[:, :].rearrange(
                            "(oh ki) n -> ki oh n", ki=128)[:, :, nsl],
                        in_=osb,
                    )

                tail_pend = None  # (pvsb, bc, nb)
                for nb in range(nblk_run):
                    pv0 = psPV.tile([128, NB], f32, tag="pv")
                    pv1 = psPV.tile([128, NB], f32, tag="pv")
                    pend[nb] = (pv0, pv1, {})
                    for ch in range(NCHUNK + PV_LAG):
                        if ch < NCHUNK:
                            pend[nb][2][ch] = emit_scores(nb, ch)
                        if ch == 3 and tail_pend is not None:
                            pvsb_p, bc_p, nb_p = tail_pend
                            proj_pend = (emit_at2(nb_p, pvsb_p, bc_p), nb_p)
                            tail_pend = None
                        if ch == 6 and proj_pend is not None:
                            emit_proj(*proj_pend)
                            proj_pend = None
                        if ch >= PV_LAG:
                            c2 = ch - PV_LAG
                            emit_pv(nb, c2, pend[nb][2].pop(c2))
                    pvsb, bc = emit_tail(nb)
                    tail_pend = (pvsb, bc, nb)
                    del pend[nb]
                if tail_pend is not None:
                    pvsb_p, bc_p, nb_p = tail_pend
                    proj_pend = (emit_at2(nb_p, pvsb_p, bc_p), nb_p)
                    tail_pend = None
                if proj_pend is not None:
                    emit_proj(*proj_pend)
                    proj_pend = None
    return nc


def split_multiwaits(nc):
    """The staged walrus accepts at most one sync-wait per instruction; Tile
    emits several. Hoist all but one wait onto same-engine NOPs placed just
    before the instruction (engine program order makes this equivalent)."""
    import concourse.mybir as mybir

    n = 0
    for fn in nc.m.functions:
        for blk in fn.blocks:
            new = []
            for inst in blk.instructions:
                si = getattr(inst, "sync_info", None)
                waits = list(si.on_wait) if si is not None and si.on_wait else []
                if len(waits) > 1:
                    for k, w in enumerate(waits[:-1]):
                        new.append(mybir.InstNoOp(
                            name=f"{inst.name}-w{k}",
                            engine=inst.engine,
                            ins=[], outs=[],
                            sync_info=mybir.SyncInfo(on_wait=[w], on_update=[]),
                        ))
                        n += 1
                    inst.sync_info = mybir.SyncInfo(
                        on_wait=[waits[-1]], on_update=list(si.on_update)
                    )
                new.append(inst)
            blk.instructions = new
    return n


def _get_nc():
    if "nc" not in _NC_CACHE:
        nc = build_nc()
        split_multiwaits(nc)
        _NC_CACHE["nc"] = nc
    return _NC_CACHE["nc"]


def _make_in_maps(x, wq, bq, wk, wv, wp):
    bf = ml_dtypes.bfloat16
    in_maps = []
    for core in range(8):
        b = core // 2
        hp = core % 2
        rs = slice(hp * DD, (hp + 1) * DD)
        in_maps.append({
            "x": np.ascontiguousarray(x[b]).astype(bf),
            "wqT": np.ascontiguousarray(wq[rs, :].T).astype(bf),
            "wkT": np.ascontiguousarray(wk[rs, :].T).astype(bf),
            "wvT": np.ascontiguousarray(wv[rs, :].T).astype(bf),
            "wpT": np.stack(
                [np.stack([np.ascontiguousarray(
                    wp[j * DD:(j + 1) * DD, rs].T[hp * 64:(hp + 1) * 64, :])
                    for j in range(2)]) for hp in range(2)]
            ).astype(bf),
            "bq2": np.ascontiguousarray(bq[rs]).reshape(DD, 1).astype(np.float32),
        })
    return in_maps


def run(x, wq, bq, wk, bk, wv, bv, wp, bp, trace=False):
    from concourse.bass_utils import run_bass_kernel_spmd

    x = np.asarray(x, dtype=np.float32)
    wq = np.asarray(wq, dtype=np.float32)
    bq = np.asarray(bq, dtype=np.float32)
    wk = np.asarray(wk, dtype=np.float32)
    wv = np.asarray(wv, dtype=np.float32)
    bv = np.asarray(bv, dtype=np.float32)
    wp = np.asarray(wp, dtype=np.float32)
    bp = np.asarray(bp, dtype=np.float32)

    nc = _get_nc()
    in_maps = _make_in_maps(x, wq, bq, wk, wv, wp)
    res = None
    for attempt in range(3):
        try:
            res = run_bass_kernel_spmd(
                nc, in_maps, core_ids=list(range(8)), trace=trace)
            break
        except Exception:
            if attempt == 2:
                raise
    parts = [r["out"].astype(np.float32) for r in res.results]

    const = (bp + wp @ bv).astype(np.float32)[:, None]  # [C, 1]
    out = np.empty((B, C, N), dtype=np.float32)
    for b in range(B):
        out[b] = parts[2 * b] + parts[2 * b + 1] + x[b] + const
    return out, res


def kernel(**inputs):
    out, _ = run(**inputs)
    return out
